# revision 13
# baseline (speedup 1.0000x reference)
"""ConceptNet encoder kernel for 8 Trainium2 NeuronCores (Bass/Tile).

Reference computation:
    emb    = table[tok]                      # [1024, 256]
    logits = emb @ table.T                   # [1024, 100000]
    idx    = top16(softmax(logits))          # softmax monotonic -> top16(logits)
    h      = table[idx]                      # [1024, 16, 256]
    e      = tanh(h @ a) @ b                 # [1024, 16]
    out    = softmax(e) @ h                  # [1024, 256]

Distribution: vocab sharded 8 ways (12500 rows/core), all 1024 tokens
scored on every core, token-sharded merge/attention (128 tokens/core).

The similarity matmul runs in bf16 (logit err sigma ~1e-4, far below
top-16 gaps ~8e-4). Selection: each PSUM chunk is quantized straight
into the HIGH fp16 halves of pre-iota'd uint32 words (one strided
activation per chunk; chunk A on the scalar engine, chunk B on
gpsimd) so each word is (fp16(QSCALE*logit+QBIAS) << 16) | slot16 —
totally ordered as fp32 with the slot as tiebreak.  One DVE MAX8 per
4096-wide group then yields the top-8 (value+slot packed) with zero
extra passes.  A barrier collective gates the key-iota so all cores
enter the (scalar/DVE-paced) main loop aligned; four AllToAll's then
pipeline with compute and the final one completes in ~2us.  Each core
merges a 256-candidate pool for its own 128 tokens, takes top-20,
re-scores the pool exactly in fp32 (rescue), and runs masked-softmax
attention (bf16 matmuls, streamed right behind the h-gathers through
a ring of 1-bank PSUM tiles) so exactly the true top-16 get weight.

kernel(**inputs) takes FULL unsharded inputs, returns FULL [4,256,256] output.
Self-contained: hardcodes all shapes; imports only the system concourse repo.
"""
import os
import sys

if "/opt/trn_rl_repo" not in sys.path:
    sys.path.insert(0, "/opt/trn_rl_repo")

import numpy as np
import ml_dtypes

import concourse.bass as bass
import concourse.bacc as bacc
import concourse.mybir as mybir
import concourse.tile as tile
from concourse import bass_utils
from concourse.masks import make_identity

DT = mybir.dt
ALU = mybir.AluOpType
ACT = mybir.ActivationFunctionType

B, L, V, E, TOPK = 4, 256, 100000, 256, 16
NCORES = 8
NTOK = B * L                 # 1024
TPC = NTOK // NCORES         # 128 tokens per core (merge/attention shard)
VS = V // NCORES             # 12500 vocab rows per core
P = 128
NEG = -3.0e38

CW = 2048                    # psum chunk width (4 banks)
CHUNKS = [(0, 2048), (2048, 2048), (4096, 2048), (6144, 2048),
          (8192, 2048), (10240, 2048), (12288, 212)]
GROUPS = [[0, 1], [2, 3], [4, 5], [6]]
GRP_OFF = [0, 4096, 8192, 12288]
GRP_W = [4096, 4096, 4096, 212]
NGRP = 4
KEYW = 4096                  # keys tile width
NCP = 8                      # candidates per (core, group)
MERGEW = NGRP * NCORES * NCP  # 256
KP = 20                      # rescue pool size per token
KPAD = 24                    # padded pool for max8 rounds
NGR = KP // 4                # attention 512-wide groups
QSCALE = 1638.4              # fp16 key quantizer: step 6.1e-4 in [1024,2048)
QBIAS = 1024.0

_BUILD_CACHE = {}
LAST_RESULTS = None


def _build():
    nc = bacc.Bacc("TRN2", target_bir_lowering=False, debug=False,
                   enable_asserts=True, num_devices=NCORES)

    tokidx = nc.dram_tensor("tokidx", [NTOK, 1], DT.int32, kind="ExternalInput").ap()
    tok_own = nc.dram_tensor("tok_own", [TPC, 1], DT.int32, kind="ExternalInput").ap()
    table = nc.dram_tensor("table", [V, E], DT.float32, kind="ExternalInput").ap()
    tabTb = nc.dram_tensor("tabTb", [E, VS], DT.bfloat16, kind="ExternalInput").ap()
    amat = nc.dram_tensor("amat", [E, E], DT.float32, kind="ExternalInput").ap()
    bvec = nc.dram_tensor("bvec", [E, 1], DT.float32, kind="ExternalInput").ap()
    out = nc.dram_tensor("out", [TPC, E], DT.float32, kind="ExternalOutput").ap()

    with tile.TileContext(nc) as tc:
        with tc.tile_pool(name="const", bufs=1) as cpool, \
             tc.tile_pool(name="big", bufs=1) as big, \
             tc.tile_pool(name="work", bufs=2) as work, \
             tc.tile_pool(name="dram", bufs=1, space="DRAM") as dram:

            # ---------------- token index DMAs first (tiny) --------------
            tis = []
            for m in range(NCORES):
                ti = work.tile([P, 1], DT.int32, tag="ti", bufs=8, name="ti")
                nc.sync.dma_start(out=ti, in_=tokidx[m * P:(m + 1) * P, :])
                tis.append(ti)
            ti_own = cpool.tile([P, 1], DT.int32, tag="ti_own")
            nc.sync.dma_start(out=ti_own, in_=tok_own)

            # ---------------- strips (bf16, full residency), group order --
            strip = [big.tile([P, VS], DT.bfloat16, tag=f"strip{kb}",
                              name=f"strip{kb}") for kb in range(2)]
            for g in range(NGRP):
                go, gw = GRP_OFF[g], GRP_W[g]
                for kb in range(2):
                    nc.sync.dma_start(out=strip[kb][:, go:go + gw],
                                      in_=tabTb[kb * P:(kb + 1) * P, go:go + gw])

            # ---------------- identity + gathers on gpsimd ----------------
            ident = cpool.tile([P, P], DT.float32, tag="ident")
            make_identity(nc, ident)

            em_tiles = []
            for m in range(NCORES):
                em = work.tile([P, E], DT.float32, tag="em", bufs=8, name="em")
                em_tiles.append(em)
            for m in range(NCORES):
                nc.gpsimd.indirect_dma_start(
                    out=em_tiles[m], out_offset=None, in_=table,
                    in_offset=bass.IndirectOffsetOnAxis(ap=tis[m][:, :], axis=0))
            emb_own = cpool.tile([P, E], DT.float32, tag="emb_own")
            nc.gpsimd.indirect_dma_start(
                out=emb_own, out_offset=None, in_=table,
                in_offset=bass.IndirectOffsetOnAxis(ap=ti_own[:, :], axis=0))

            def const_col(name, val):
                t = cpool.tile([P, 1], DT.uint32, tag=name, name=name)
                nc.gpsimd.iota(t, pattern=[[0, 1]], base=val, channel_multiplier=0)
                return t

            c_mask16 = const_col("c_mask16", 0xFFFF)
            c_6 = const_col("c_6", 6)
            c_3 = const_col("c_3", 3)
            c_7 = const_col("c_7", 7)

            # barrier: aligns cores (keys iota — and so the whole
            # scalar/DVE-paced main loop — waits for it; compute queues
            # keep running on already-issued work meanwhile)
            bar = dram.tile([NCORES, 1], DT.float32, tag="bar", name="bar")
            bar2 = dram.tile([NCORES, 1], DT.float32, tag="bar2", name="bar2")
            nc.gpsimd.collective_compute(
                "AllToAll", ALU.bypass,
                replica_groups=[list(range(NCORES))],
                ins=[bar[:, :].opt()], outs=[bar2[:, :].opt()])

            # keys tiles: lo halves = slot iota (persist), hi = fp16 keys
            keysT = [big.tile([P, KEYW], DT.uint32, tag=f"keys{r}",
                              name=f"keys{r}") for r in range(2)]
            nc.gpsimd.iota(keysT[0][:, :2048], pattern=[[1, 2048]], base=0,
                           channel_multiplier=0)
            nc.vector.tensor_scalar(keysT[0][:, 2048:], keysT[0][:, :2048],
                                    2048.0, None, op0=ALU.add)
            nc.vector.tensor_copy(keysT[1], keysT[0])

            # preload the scalar-engine activation table (tanh/exp) AND gate
            # the scalar queue (which paces the main loop) on the barrier so
            # all cores enter the loop aligned: bar2 -> DMA -> DVE -> tanh.
            warm = cpool.tile([P, 8], DT.float32, tag="warm")
            nc.vector.memset(warm, 0.0)
            bt = cpool.tile([NCORES, 1], DT.float32, tag="bt")
            nc.sync.dma_start(out=bt, in_=bar2)
            nc.vector.tensor_scalar(warm[0:NCORES, 0:1], bt, 0.0, None,
                                    op0=ALU.mult)
            nc.scalar.activation(warm, warm, ACT.Tanh)

            # ---------------- small attention weights (bf16) --------------
            a_b = []
            for kb in range(2):
                t0 = work.tile([P, E], DT.float32, tag="aw")
                nc.sync.dma_start(out=t0, in_=amat[kb * P:(kb + 1) * P, :])
                t = cpool.tile([P, E], DT.bfloat16, tag=f"ab{kb}", name=f"ab{kb}")
                nc.vector.tensor_copy(t, t0)
                a_b.append(t)
            b_b = []
            for kb in range(2):
                t0 = work.tile([P, 1], DT.float32, tag="bw")
                nc.sync.dma_start(out=t0, in_=bvec[kb * P:(kb + 1) * P, :])
                t = cpool.tile([P, 1], DT.bfloat16, tag=f"bb{kb}", name=f"bb{kb}")
                nc.vector.tensor_copy(t, t0)
                b_b.append(t)

            # ---------------- a2a bounce buffers ----------------
            bounce = [dram.tile([NCORES, TPC, NCP], DT.float32, tag=f"bounce{g}",
                                name=f"bounce{g}")
                      for g in range(NGRP)]
            agg = [dram.tile([NCORES * TPC * NCP, 1], DT.float32, tag=f"agg{g}",
                             name=f"agg{g}")
                   for g in range(NGRP)]
            scd = dram.tile([1, TPC * KP], DT.float32, tag="scd")

            vals = cpool.tile([P, MERGEW], DT.float32, tag="vals")

            def load_vals(g):
                # vals[p, g*64 + c*8 + s] = agg[g][(c, p, s)]
                agg_v = agg[g][:, :].rearrange("(c p s) o -> c p (s o)",
                                               c=NCORES, p=TPC).transpose([1, 0, 2])
                out_v = vals[:, g * 64:(g + 1) * 64].rearrange(
                    "p (c s) -> p c s", c=NCORES)
                nc.sync.dma_start(out=out_v, in_=agg_v)

            def a2a(g):
                nc.gpsimd.collective_compute(
                    "AllToAll", ALU.bypass,
                    replica_groups=[list(range(NCORES))],
                    ins=[bounce[g][:, :, :].opt()],
                    outs=[agg[g][:, :].opt()],
                )

            embT = [[big.tile([P, P], DT.bfloat16, tag=f"embT{kb}_{m}",
                              name=f"embT{kb}_{m}")
                     for m in range(NCORES)] for kb in range(2)]

            # ============ main pipeline: psum pool scope =================
            with tc.tile_pool(name="ps", bufs=2, space="PSUM") as ps:
                # transposes + bf16 casts of emb
                for m in range(NCORES):
                    pt = ps.tile([P, CW], DT.float32, tag="chunk", name="pt")
                    for kb in range(2):
                        nc.tensor.transpose(out=pt[:, kb * P:(kb + 1) * P],
                                            in_=em_tiles[m][:, kb * P:(kb + 1) * P],
                                            identity=ident)
                    for kb in range(2):
                        nc.vector.tensor_copy(embT[kb][m],
                                              pt[:, kb * P:(kb + 1) * P])

                # similarity + per-group packed top-8
                for g in range(NGRP):
                    go, gw = GRP_OFF[g], GRP_W[g]
                    for m in range(NCORES):
                        if m == 1 and g >= 1:
                            a2a(g - 1)
                        if m == 5 and g >= 2:
                            load_vals(g - 2)
                        kr = keysT[m & 1]
                        pss = []
                        for ci in GROUPS[g]:
                            off, w = CHUNKS[ci]
                            pchunk = ps.tile([P, CW], DT.float32, tag="chunk",
                                             name="pchunk")
                            pss.append((pchunk, off, w))
                        for kb in range(2):
                            for pt, off, w in pss:
                                for h in range(0, w, 512):
                                    hw = min(512, w - h)
                                    nc.tensor.matmul(
                                        pt[:, h:h + hw], embT[kb][m],
                                        strip[kb][:, off + h:off + h + hw],
                                        start=(kb == 0), stop=(kb == 1))
                        for pt, off, w in pss:
                            lo = off - go
                            hi16 = kr.bitcast(DT.float16).rearrange(
                                "p (w two) -> p two w", two=2)[:, 1, lo:lo + w]
                            nc.scalar.activation(hi16, pt[:, :w], ACT.Copy,
                                                 scale=QSCALE, bias=QBIAS)
                        cv = work.tile([P, NCP], DT.float32, tag="cv", bufs=4)
                        nc.vector.max(out=cv, in_=kr.bitcast(DT.float32)[:, :gw])
                        nc.sync.dma_start(out=bounce[g][m, :, :], in_=cv)

                a2a(NGRP - 1)
                load_vals(NGRP - 2)
                load_vals(NGRP - 1)

            # ============ merge / rescue / attention: 1-bank psum ========
            wk = cpool.tile([P, KPAD], DT.float32, tag="wk")
            wp = cpool.tile([P, KPAD], DT.uint32, tag="wp")
            vals2 = cpool.tile([P, MERGEW], DT.float32, tag="vals2")
            vals3 = cpool.tile([P, MERGEW], DT.float32, tag="vals3")
            slot = cpool.tile([P, KPAD], DT.uint32, tag="slot", name="slot")
            grp = cpool.tile([P, KPAD], DT.uint32, tag="grp", name="grp")
            csrc = cpool.tile([P, KPAD], DT.uint32, tag="csrc", name="csrc")
            gidx = cpool.tile([P, KPAD], DT.uint32, tag="gidx", name="gidx")
            t2 = cpool.tile([P, KPAD], DT.uint32, tag="t2", name="t2")
            hk = [cpool.tile([P, E], DT.float32, tag=f"h{k}", name=f"h{k}")
                  for k in range(KP)]
            hTs = [[big.tile([P, 512], DT.bfloat16, tag=f"hTs{gA}_{kb}",
                             name=f"hTs{gA}_{kb}") for kb in range(2)]
                   for gA in range(NGR)]
            tanhTs = [[big.tile([P, 512], DT.bfloat16, tag=f"tanhTs{gA}_{eb}",
                                name=f"tanhTs{gA}_{eb}") for eb in range(2)]
                      for gA in range(NGR)]
            d = cpool.tile([P, KPAD], DT.float32, tag="d")
            nc.vector.memset(d[:, KP:], NEG)
            prod = cpool.tile([P, E], DT.float32, tag="prod", bufs=2)

            with tc.tile_pool(name="ps2", bufs=1, space="PSUM") as ps2:

                def decode_and_gather(g0, g1):
                    gs = slice(g0, g1)
                    nc.vector.tensor_scalar(slot[:, gs],
                                            wk[:, gs].bitcast(DT.uint32),
                                            c_mask16[:, :], None,
                                            op0=ALU.bitwise_and)
                    nc.vector.tensor_scalar(grp[:, gs], wp[:, gs], c_6[:, :],
                                            None, op0=ALU.logical_shift_right)
                    nc.vector.tensor_scalar(csrc[:, gs], wp[:, gs], c_3[:, :],
                                            None, op0=ALU.logical_shift_right)
                    nc.vector.tensor_scalar(csrc[:, gs], csrc[:, gs], c_7[:, :],
                                            None, op0=ALU.bitwise_and)
                    nc.vector.tensor_scalar(gidx[:, gs], csrc[:, gs], float(VS),
                                            None, op0=ALU.mult)
                    nc.vector.tensor_scalar(t2[:, gs], grp[:, gs], 4096.0, None,
                                            op0=ALU.mult)
                    nc.vector.tensor_tensor(gidx[:, gs], gidx[:, gs], t2[:, gs],
                                            op=ALU.add)
                    nc.vector.tensor_tensor(gidx[:, gs], gidx[:, gs], slot[:, gs],
                                            op=ALU.add)
                    for k in range(g0, min(g1, KP)):
                        nc.gpsimd.indirect_dma_start(
                            out=hk[k], out_offset=None, in_=table,
                            in_offset=bass.IndirectOffsetOnAxis(
                                ap=gidx[:, :].bitcast(DT.int32)[:, k:k + 1],
                                axis=0))

                def dots(k0, k1):
                    for k in range(k0, min(k1, KP)):
                        nc.vector.scalar_tensor_tensor(
                            prod, hk[k], 1.0, emb_own,
                            op0=ALU.mult, op1=ALU.mult, accum_out=d[:, k:k + 1])

                def transpose_pair(k0):
                    # transpose hk[k0], hk[k0+1] into one 1-bank psum tile
                    ptt = ps2.tile([P, 512], DT.float32, tag="ptr", name="ptt",
                                   bufs=3)
                    for j in range(2):
                        k = k0 + j
                        for kb in range(2):
                            nc.tensor.transpose(
                                out=ptt[:, (j * 2 + kb) * P:(j * 2 + kb + 1) * P],
                                in_=hk[k][:, kb * P:(kb + 1) * P],
                                identity=ident)
                    for j in range(2):
                        k = k0 + j
                        gA, kk = k // 4, k % 4
                        nc.vector.tensor_copy(
                            hTs[gA][0][:, kk * P:(kk + 1) * P],
                            ptt[:, (j * 2) * P:(j * 2 + 1) * P])
                        nc.scalar.activation(
                            hTs[gA][1][:, kk * P:(kk + 1) * P],
                            ptt[:, (j * 2 + 1) * P:(j * 2 + 2) * P], ACT.Copy)

                def attn_group(gA):
                    for eb in range(2):
                        pta = ps2.tile([P, 512], DT.float32, tag="pta",
                                       name="pta", bufs=3)
                        for kb in range(2):
                            nc.tensor.matmul(pta, a_b[kb][:, eb * P:(eb + 1) * P],
                                             hTs[gA][kb], start=(kb == 0),
                                             stop=(kb == 1))
                        nc.scalar.activation(tanhTs[gA][eb], pta, ACT.Tanh)
                    psc = ps2.tile([P, 512], DT.float32, tag="psc", name="psc",
                                   bufs=2)
                    for eb in range(2):
                        nc.tensor.matmul(psc[:1, :], b_b[eb], tanhTs[gA][eb],
                                         start=(eb == 0), stop=(eb == 1))
                    scs = work.tile([1, 512], DT.float32, tag="scs", bufs=3)
                    nc.vector.tensor_copy(scs, psc[:1, :])
                    nc.sync.dma_start(out=scd[:, gA * 512:(gA + 1) * 512], in_=scs)

                nc.vector.max(out=wk[:, 0:8], in_=vals)
                nc.vector.max_index(out=wp[:, 0:8], in_max=wk[:, 0:8],
                                    in_values=vals)
                nc.vector.match_replace(out=vals2, in_to_replace=wk[:, 0:8],
                                        in_values=vals, imm_value=0.0)
                decode_and_gather(0, 8)
                nc.vector.max(out=wk[:, 8:16], in_=vals2)
                nc.vector.max_index(out=wp[:, 8:16], in_max=wk[:, 8:16],
                                    in_values=vals2)
                nc.vector.match_replace(out=vals3, in_to_replace=wk[:, 8:16],
                                        in_values=vals2, imm_value=0.0)
                decode_and_gather(8, 16)
                nc.vector.max(out=wk[:, 16:24], in_=vals3)
                nc.vector.max_index(out=wp[:, 16:24], in_max=wk[:, 16:24],
                                    in_values=vals3)
                decode_and_gather(16, KP)

                dots(0, 8)
                for k0 in (0, 2, 4, 6):
                    transpose_pair(k0)
                attn_group(0)
                attn_group(1)
                dots(8, 16)
                for k0 in (8, 10, 12, 14):
                    transpose_pair(k0)
                attn_group(2)
                attn_group(3)
                dots(16, KP)
                for k0 in (16, 18):
                    transpose_pair(k0)
                attn_group(4)

                # 16th largest exact dot -> threshold mask
                t8a = cpool.tile([P, 8], DT.float32, tag="t8a")
                t8b = cpool.tile([P, 8], DT.float32, tag="t8b")
                d2 = cpool.tile([P, KPAD], DT.float32, tag="d2")
                nc.vector.max(out=t8a, in_=d)
                nc.vector.match_replace(out=d2, in_to_replace=t8a, in_values=d,
                                        imm_value=NEG)
                nc.vector.max(out=t8b, in_=d2)
                maskp = cpool.tile([P, KP], DT.float32, tag="maskp")
                nc.vector.tensor_scalar(maskp, d[:, :KP], t8b[:, 7:8], None,
                                        op0=ALU.is_ge)
                nc.vector.tensor_scalar(maskp, maskp, -1.0, 1.0e9,
                                        op0=ALU.add, op1=ALU.mult)

                # scores [t, k] <- scd[k*128 + t]; per-group unnormalized
                # softmax-accumulate (scores bounded: no max-sub needed)
                sct = cpool.tile([P, KP], DT.float32, tag="sct")
                exv = cpool.tile([P, KP], DT.float32, tag="exv")
                acc = cpool.tile([P, E], DT.float32, tag="acc")
                acc2 = cpool.tile([P, E], DT.float32, tag="acc2")
                nc.vector.memset(acc, 0.0)
                nc.vector.memset(acc2, 0.0)
                for gA in range(NGR):
                    gs = slice(gA * 4, (gA + 1) * 4)
                    nc.sync.dma_start(
                        out=sct[:, gs],
                        in_=scd[:, gA * 512:(gA + 1) * 512].rearrange(
                            "o (k t) -> (o t) k", t=TPC))
                    nc.vector.tensor_tensor(sct[:, gs], sct[:, gs],
                                            maskp[:, gs], op=ALU.add)
                    nc.scalar.activation(exv[:, gs], sct[:, gs], ACT.Exp)
                    for k in range(gA * 4, (gA + 1) * 4):
                        ac = acc if k % 2 == 0 else acc2
                        nc.vector.scalar_tensor_tensor(
                            ac, hk[k], exv[:, k:k + 1], ac,
                            op0=ALU.mult, op1=ALU.add)
                sm = cpool.tile([P, 1], DT.float32, tag="sm")
                nc.vector.reduce_sum(sm, exv, axis=mybir.AxisListType.X)
                rc = cpool.tile([P, 1], DT.float32, tag="rc")
                nc.vector.reciprocal(rc, sm)
                nc.vector.tensor_tensor(acc, acc, acc2, op=ALU.add)
                nc.vector.tensor_scalar(acc, acc, rc[:, :], None, op0=ALU.mult)
                nc.sync.dma_start(out=out, in_=acc)

    nc.compile()
    return nc


def get_nc():
    if "v6" not in _BUILD_CACHE:
        _BUILD_CACHE["v6"] = _build()
    return _BUILD_CACHE["v6"]


def kernel(conceptnet_text_vec, table, a, b, topk=16, **_ignored):
    global LAST_RESULTS
    assert int(topk) == TOPK
    tok = np.asarray(conceptnet_text_vec).reshape(NTOK, 1).astype(np.int32)
    table = np.ascontiguousarray(np.asarray(table, dtype=np.float32))
    a = np.ascontiguousarray(np.asarray(a, dtype=np.float32))
    b = np.ascontiguousarray(np.asarray(b, dtype=np.float32)).reshape(E, 1)
    tabT = np.ascontiguousarray(table.T)     # [E, V]

    nc = get_nc()
    in_maps = []
    for c in range(NCORES):
        in_maps.append({
            "tokidx": tok,
            "tok_own": np.ascontiguousarray(tok[c * TPC:(c + 1) * TPC]),
            "table": table,
            "tabTb": np.ascontiguousarray(
                tabT[:, c * VS:(c + 1) * VS]).astype(ml_dtypes.bfloat16),
            "amat": a,
            "bvec": b,
        })
    trace = bool(int(os.environ.get("CN_TRACE", "0")))
    res = bass_utils.run_bass_kernel_spmd(nc, in_maps, core_ids=list(range(NCORES)),
                                          trace=trace)
    LAST_RESULTS = res
    outp = np.concatenate([res.results[c]["out"] for c in range(NCORES)], axis=0)
    return outp.reshape(B, L, E)


# revision 18
# speedup vs baseline: 1.1957x; 1.1957x over previous
"""ConceptNet encoder kernel for 8 Trainium2 NeuronCores (Bass/Tile).

Reference computation:
    emb    = table[tok]                      # [1024, 256]
    logits = emb @ table.T                   # [1024, 100000]
    idx    = top16(softmax(logits))          # softmax monotonic -> top16(logits)
    h      = table[idx]                      # [1024, 16, 256]
    e      = tanh(h @ a) @ b                 # [1024, 16]
    out    = softmax(e) @ h                  # [1024, 256]

Distribution: vocab sharded 8 ways (12500 rows/core), all 1024 tokens
scored on every core, token-sharded merge/attention (128 tokens/core).

The similarity matmul runs in bf16 (logit err sigma ~1e-4, far below
top-16 gaps ~8e-4). Selection: each PSUM chunk is quantized straight
into the HIGH fp16 halves of pre-iota'd uint32 words (one strided
activation per chunk; chunk A on the scalar engine, chunk B on
gpsimd) so each word is (fp16(QSCALE*logit+QBIAS) << 16) | slot16 —
totally ordered as fp32 with the slot as tiebreak.  One DVE MAX8 per
4096-wide group then yields the top-8 (value+slot packed) with zero
extra passes.  A barrier collective gates the key-iota so all cores
enter the (scalar/DVE-paced) main loop aligned; four AllToAll's then
pipeline with compute and the final one completes in ~2us.  Each core
merges a 256-candidate pool for its own 128 tokens, takes top-20,
re-scores the pool exactly in fp32 (rescue), and runs masked-softmax
attention (bf16 matmuls, streamed right behind the h-gathers through
a ring of 1-bank PSUM tiles) so exactly the true top-16 get weight.

kernel(**inputs) takes FULL unsharded inputs, returns FULL [4,256,256] output.
Self-contained: hardcodes all shapes; imports only the system concourse repo.
"""
import os
import sys

if "/opt/trn_rl_repo" not in sys.path:
    sys.path.insert(0, "/opt/trn_rl_repo")

import numpy as np
import ml_dtypes

import concourse.bass as bass
import concourse.bacc as bacc
import concourse.mybir as mybir
import concourse.tile as tile
from concourse import bass_utils
from concourse.masks import make_identity

DT = mybir.dt
ALU = mybir.AluOpType
ACT = mybir.ActivationFunctionType

B, L, V, E, TOPK = 4, 256, 100000, 256, 16
NCORES = 8
NTOK = B * L                 # 1024
TPC = NTOK // NCORES         # 128 tokens per core (merge/attention shard)
VS = V // NCORES             # 12500 vocab rows per core
P = 128
NEG = -3.0e38

CW = 2048                    # psum chunk width (4 banks)
CHUNKS = [(0, 2048), (2048, 2048), (4096, 2048), (6144, 2048),
          (8192, 2048), (10240, 2048), (12288, 212)]
GROUPS = [[0, 1], [2, 3], [4, 5], [6]]
GRP_OFF = [0, 4096, 8192, 12288]
GRP_W = [4096, 4096, 4096, 212]
NGRP = 4
KEYW = 4096                  # keys tile width
NCP = 8                      # candidates per (core, group)
MERGEW = NGRP * NCORES * NCP  # 256
KP = 20                      # rescue pool size per token
KPAD = 24                    # padded pool for max8 rounds
NGR = KP // 4                # attention 512-wide groups
QSCALE = 1638.4              # fp16 key quantizer: step 6.1e-4 in [1024,2048)
QBIAS = 1024.0

_BUILD_CACHE = {}
LAST_RESULTS = None


def _build():
    nc = bacc.Bacc("TRN2", target_bir_lowering=False, debug=False,
                   enable_asserts=True, num_devices=NCORES)

    tokidx = nc.dram_tensor("tokidx", [NTOK, 1], DT.int32, kind="ExternalInput").ap()
    tok_own = nc.dram_tensor("tok_own", [TPC, 1], DT.int32, kind="ExternalInput").ap()
    table = nc.dram_tensor("table", [V, E], DT.float32, kind="ExternalInput").ap()
    tabTb = nc.dram_tensor("tabTb", [E, VS], DT.bfloat16, kind="ExternalInput").ap()
    amat = nc.dram_tensor("amat", [E, E], DT.float32, kind="ExternalInput").ap()
    bvec = nc.dram_tensor("bvec", [E, 1], DT.float32, kind="ExternalInput").ap()
    out = nc.dram_tensor("out", [TPC, E], DT.float32, kind="ExternalOutput").ap()

    with tile.TileContext(nc) as tc:
        with tc.tile_pool(name="const", bufs=1) as cpool, \
             tc.tile_pool(name="big", bufs=1) as big, \
             tc.tile_pool(name="work", bufs=2) as work, \
             tc.tile_pool(name="dram", bufs=1, space="DRAM") as dram:

            # ---------------- token index DMAs first (tiny) --------------
            tis = []
            for m in range(NCORES):
                ti = work.tile([P, 1], DT.int32, tag="ti", bufs=8, name="ti")
                nc.sync.dma_start(out=ti, in_=tokidx[m * P:(m + 1) * P, :])
                tis.append(ti)
            ti_own = cpool.tile([P, 1], DT.int32, tag="ti_own")
            nc.sync.dma_start(out=ti_own, in_=tok_own)

            # ---------------- strips (bf16, full residency), group order --
            strip = [big.tile([P, VS], DT.bfloat16, tag=f"strip{kb}",
                              name=f"strip{kb}") for kb in range(2)]
            for g in range(NGRP):
                go, gw = GRP_OFF[g], GRP_W[g]
                for kb in range(2):
                    nc.sync.dma_start(out=strip[kb][:, go:go + gw],
                                      in_=tabTb[kb * P:(kb + 1) * P, go:go + gw])

            # ---------------- identity + keys iota first on gpsimd --------
            ident = cpool.tile([P, P], DT.float32, tag="ident")
            make_identity(nc, ident)

            # keys tiles: lo halves = slot iota (persist), hi = fp16 keys
            keysT = [big.tile([P, KEYW], DT.uint32, tag=f"keys{r}",
                              name=f"keys{r}") for r in range(2)]
            nc.gpsimd.iota(keysT[0][:, :2048], pattern=[[1, 2048]], base=0,
                           channel_multiplier=0)
            nc.vector.tensor_scalar(keysT[0][:, 2048:], keysT[0][:, :2048],
                                    2048.0, None, op0=ALU.add)
            nc.vector.tensor_copy(keysT[1], keysT[0])

            em_tiles = []
            for m in range(NCORES):
                em = work.tile([P, E], DT.float32, tag="em", bufs=8, name="em")
                em_tiles.append(em)
            for m in range(NCORES):
                nc.gpsimd.indirect_dma_start(
                    out=em_tiles[m], out_offset=None, in_=table,
                    in_offset=bass.IndirectOffsetOnAxis(ap=tis[m][:, :], axis=0))
            emb_own = cpool.tile([P, E], DT.float32, tag="emb_own")
            nc.gpsimd.indirect_dma_start(
                out=emb_own, out_offset=None, in_=table,
                in_offset=bass.IndirectOffsetOnAxis(ap=ti_own[:, :], axis=0))

            def const_col(name, val):
                t = cpool.tile([P, 1], DT.uint32, tag=name, name=name)
                nc.gpsimd.iota(t, pattern=[[0, 1]], base=val, channel_multiplier=0)
                return t

            c_mask16 = const_col("c_mask16", 0xFFFF)
            c_5 = const_col("c_5", 5)
            c_3 = const_col("c_3", 3)
            c_m3 = const_col("c_m3", 3)

            # preload the scalar-engine activation table (tanh/exp)
            warm = cpool.tile([P, 8], DT.float32, tag="warm")
            nc.vector.memset(warm, 0.0)
            nc.scalar.activation(warm, warm, ACT.Tanh)

            # ---------------- small attention weights (bf16) --------------
            a_b = []
            for kb in range(2):
                t0 = work.tile([P, E], DT.float32, tag="aw")
                nc.sync.dma_start(out=t0, in_=amat[kb * P:(kb + 1) * P, :])
                t = cpool.tile([P, E], DT.bfloat16, tag=f"ab{kb}", name=f"ab{kb}")
                nc.vector.tensor_copy(t, t0)
                a_b.append(t)
            b_b = []
            for kb in range(2):
                t0 = work.tile([P, 1], DT.float32, tag="bw")
                nc.sync.dma_start(out=t0, in_=bvec[kb * P:(kb + 1) * P, :])
                t = cpool.tile([P, 1], DT.bfloat16, tag=f"bb{kb}", name=f"bb{kb}")
                nc.vector.tensor_copy(t, t0)
                b_b.append(t)

            # ---------------- single merged a2a bounce buffer -------------
            bounceA = dram.tile([NCORES, TPC, NGRP * NCP], DT.float32,
                                tag="bounceA", name="bounceA")
            aggA = dram.tile([NCORES * TPC * NGRP * NCP, 1], DT.float32,
                             tag="aggA", name="aggA")
            scd = dram.tile([1, TPC * KP], DT.float32, tag="scd")

            vals = cpool.tile([P, MERGEW], DT.float32, tag="vals")

            def load_vals():
                # vals[p, c*32 + g*8 + s] = aggA[(c, p, g*8+s)]
                agg_v = aggA[:, :].rearrange(
                    "(c p s) o -> c p (s o)", c=NCORES,
                    p=TPC).transpose([1, 0, 2])
                out_v = vals.rearrange("p (c s) -> p c s", c=NCORES)
                nc.sync.dma_start(out=out_v, in_=agg_v)

            def a2a_all():
                nc.gpsimd.collective_compute(
                    "AllToAll", ALU.bypass,
                    replica_groups=[list(range(NCORES))],
                    ins=[bounceA[:, :, :].opt()],
                    outs=[aggA[:, :].opt()],
                )

            embT = [[big.tile([P, P], DT.bfloat16, tag=f"embT{kb}_{m}",
                              name=f"embT{kb}_{m}")
                     for m in range(NCORES)] for kb in range(2)]

            # ============ main pipeline: psum pool scope =================
            with tc.tile_pool(name="ps", bufs=2, space="PSUM") as ps:
                # transposes + bf16 casts of emb
                for m in range(NCORES):
                    pt = ps.tile([P, CW], DT.float32, tag="chunk", name="pt")
                    for kb in range(2):
                        nc.tensor.transpose(out=pt[:, kb * P:(kb + 1) * P],
                                            in_=em_tiles[m][:, kb * P:(kb + 1) * P],
                                            identity=ident)
                    for kb in range(2):
                        nc.vector.tensor_copy(embT[kb][m],
                                              pt[:, kb * P:(kb + 1) * P])

                # similarity + per-group packed top-8
                for g in range(NGRP):
                    go, gw = GRP_OFF[g], GRP_W[g]
                    for m in range(NCORES):
                        kr = keysT[m & 1]
                        pss = []
                        for ci in GROUPS[g]:
                            off, w = CHUNKS[ci]
                            pchunk = ps.tile([P, CW], DT.float32, tag="chunk",
                                             name="pchunk")
                            pss.append((pchunk, off, w))
                        for kb in range(2):
                            for pt, off, w in pss:
                                for h in range(0, w, 512):
                                    hw = min(512, w - h)
                                    nc.tensor.matmul(
                                        pt[:, h:h + hw], embT[kb][m],
                                        strip[kb][:, off + h:off + h + hw],
                                        start=(kb == 0), stop=(kb == 1))
                        for pt, off, w in pss:
                            lo = off - go
                            hi16 = kr.bitcast(DT.float16).rearrange(
                                "p (w two) -> p two w", two=2)[:, 1, lo:lo + w]
                            nc.scalar.activation(hi16, pt[:, :w], ACT.Copy,
                                                 scale=QSCALE, bias=QBIAS)
                        cv = work.tile([P, NCP], DT.float32, tag="cv", bufs=4)
                        nc.vector.max(out=cv, in_=kr.bitcast(DT.float32)[:, :gw])
                        nc.sync.dma_start(
                            out=bounceA[m, :, g * NCP:(g + 1) * NCP], in_=cv)

                a2a_all()
                load_vals()

            # ============ merge / rescue / attention: 1-bank psum ========
            wk = cpool.tile([P, KPAD], DT.float32, tag="wk")
            wp = cpool.tile([P, KPAD], DT.uint32, tag="wp")
            vals2 = cpool.tile([P, MERGEW], DT.float32, tag="vals2")
            vals3 = cpool.tile([P, MERGEW], DT.float32, tag="vals3")
            slot = cpool.tile([P, KPAD], DT.uint32, tag="slot", name="slot")
            grp = cpool.tile([P, KPAD], DT.uint32, tag="grp", name="grp")
            csrc = cpool.tile([P, KPAD], DT.uint32, tag="csrc", name="csrc")
            gidx = cpool.tile([P, KPAD], DT.uint32, tag="gidx", name="gidx")
            t2 = cpool.tile([P, KPAD], DT.uint32, tag="t2", name="t2")
            hk = [cpool.tile([P, E], DT.float32, tag=f"h{k}", name=f"h{k}")
                  for k in range(KP)]
            hTs = [[big.tile([P, 512], DT.bfloat16, tag=f"hTs{gA}_{kb}",
                             name=f"hTs{gA}_{kb}") for kb in range(2)]
                   for gA in range(NGR)]
            tanhTs = [[big.tile([P, 512], DT.bfloat16, tag=f"tanhTs{gA}_{eb}",
                                name=f"tanhTs{gA}_{eb}") for eb in range(2)]
                      for gA in range(NGR)]
            d = cpool.tile([P, KPAD], DT.float32, tag="d")
            nc.vector.memset(d[:, KP:], NEG)
            prod = cpool.tile([P, E], DT.float32, tag="prod", bufs=2)

            with tc.tile_pool(name="ps2", bufs=1, space="PSUM") as ps2:

                def decode_and_gather(g0, g1):
                    # pos = c*32 + g*8 + r ; key = (fp16 << 16) | slot16
                    gs = slice(g0, g1)
                    nc.vector.tensor_scalar(slot[:, gs],
                                            wk[:, gs].bitcast(DT.uint32),
                                            c_mask16[:, :], None,
                                            op0=ALU.bitwise_and)
                    nc.vector.tensor_scalar(csrc[:, gs], wp[:, gs], c_5[:, :],
                                            None, op0=ALU.logical_shift_right)
                    nc.vector.tensor_scalar(grp[:, gs], wp[:, gs], c_3[:, :],
                                            None, op0=ALU.logical_shift_right)
                    nc.vector.tensor_scalar(grp[:, gs], grp[:, gs], c_m3[:, :],
                                            None, op0=ALU.bitwise_and)
                    nc.vector.tensor_scalar(gidx[:, gs], csrc[:, gs], float(VS),
                                            None, op0=ALU.mult)
                    nc.vector.tensor_scalar(t2[:, gs], grp[:, gs], 4096.0, None,
                                            op0=ALU.mult)
                    nc.vector.tensor_tensor(gidx[:, gs], gidx[:, gs], t2[:, gs],
                                            op=ALU.add)
                    nc.vector.tensor_tensor(gidx[:, gs], gidx[:, gs], slot[:, gs],
                                            op=ALU.add)
                    for k in range(g0, min(g1, KP)):
                        nc.gpsimd.indirect_dma_start(
                            out=hk[k], out_offset=None, in_=table,
                            in_offset=bass.IndirectOffsetOnAxis(
                                ap=gidx[:, :].bitcast(DT.int32)[:, k:k + 1],
                                axis=0))

                def dots(k0, k1):
                    for k in range(k0, min(k1, KP)):
                        nc.vector.scalar_tensor_tensor(
                            prod, hk[k], 1.0, emb_own,
                            op0=ALU.mult, op1=ALU.mult, accum_out=d[:, k:k + 1])

                def transpose_pair(k0):
                    # transpose hk[k0], hk[k0+1] into one 1-bank psum tile
                    ptt = ps2.tile([P, 512], DT.float32, tag="ptr", name="ptt",
                                   bufs=3)
                    for j in range(2):
                        k = k0 + j
                        for kb in range(2):
                            nc.tensor.transpose(
                                out=ptt[:, (j * 2 + kb) * P:(j * 2 + kb + 1) * P],
                                in_=hk[k][:, kb * P:(kb + 1) * P],
                                identity=ident)
                    for j in range(2):
                        k = k0 + j
                        gA, kk = k // 4, k % 4
                        nc.vector.tensor_copy(
                            hTs[gA][0][:, kk * P:(kk + 1) * P],
                            ptt[:, (j * 2) * P:(j * 2 + 1) * P])
                        nc.scalar.activation(
                            hTs[gA][1][:, kk * P:(kk + 1) * P],
                            ptt[:, (j * 2 + 1) * P:(j * 2 + 2) * P], ACT.Copy)

                def attn_group(gA):
                    for eb in range(2):
                        pta = ps2.tile([P, 512], DT.float32, tag="pta",
                                       name="pta", bufs=3)
                        for kb in range(2):
                            nc.tensor.matmul(pta, a_b[kb][:, eb * P:(eb + 1) * P],
                                             hTs[gA][kb], start=(kb == 0),
                                             stop=(kb == 1))
                        nc.scalar.activation(tanhTs[gA][eb], pta, ACT.Tanh)
                    psc = ps2.tile([P, 512], DT.float32, tag="psc", name="psc",
                                   bufs=2)
                    for eb in range(2):
                        nc.tensor.matmul(psc[:1, :], b_b[eb], tanhTs[gA][eb],
                                         start=(eb == 0), stop=(eb == 1))
                    scs = work.tile([1, 512], DT.float32, tag="scs", bufs=3)
                    nc.vector.tensor_copy(scs, psc[:1, :])
                    nc.sync.dma_start(out=scd[:, gA * 512:(gA + 1) * 512], in_=scs)

                nc.vector.max(out=wk[:, 0:8], in_=vals)
                nc.vector.max_index(out=wp[:, 0:8], in_max=wk[:, 0:8],
                                    in_values=vals)
                nc.vector.match_replace(out=vals2, in_to_replace=wk[:, 0:8],
                                        in_values=vals, imm_value=0.0)
                decode_and_gather(0, 8)
                nc.vector.max(out=wk[:, 8:16], in_=vals2)
                nc.vector.max_index(out=wp[:, 8:16], in_max=wk[:, 8:16],
                                    in_values=vals2)
                nc.vector.match_replace(out=vals3, in_to_replace=wk[:, 8:16],
                                        in_values=vals2, imm_value=0.0)
                decode_and_gather(8, 16)
                nc.vector.max(out=wk[:, 16:24], in_=vals3)
                nc.vector.max_index(out=wp[:, 16:24], in_max=wk[:, 16:24],
                                    in_values=vals3)
                decode_and_gather(16, KP)

                dots(0, 8)
                for k0 in (0, 2, 4, 6):
                    transpose_pair(k0)
                attn_group(0)
                attn_group(1)
                dots(8, 16)
                for k0 in (8, 10, 12, 14):
                    transpose_pair(k0)
                attn_group(2)
                attn_group(3)
                dots(16, KP)
                for k0 in (16, 18):
                    transpose_pair(k0)
                attn_group(4)

                # 16th largest exact dot -> threshold mask
                t8a = cpool.tile([P, 8], DT.float32, tag="t8a")
                t8b = cpool.tile([P, 8], DT.float32, tag="t8b")
                d2 = cpool.tile([P, KPAD], DT.float32, tag="d2")
                nc.vector.max(out=t8a, in_=d)
                nc.vector.match_replace(out=d2, in_to_replace=t8a, in_values=d,
                                        imm_value=NEG)
                nc.vector.max(out=t8b, in_=d2)
                maskp = cpool.tile([P, KP], DT.float32, tag="maskp")
                nc.vector.tensor_scalar(maskp, d[:, :KP], t8b[:, 7:8], None,
                                        op0=ALU.is_ge)
                nc.vector.tensor_scalar(maskp, maskp, -1.0, 1.0e9,
                                        op0=ALU.add, op1=ALU.mult)

                # scores [t, k] <- scd[k*128 + t]; per-group unnormalized
                # softmax-accumulate (scores bounded: no max-sub needed)
                sct = cpool.tile([P, KP], DT.float32, tag="sct")
                exv = cpool.tile([P, KP], DT.float32, tag="exv")
                acc = cpool.tile([P, E], DT.float32, tag="acc")
                acc2 = cpool.tile([P, E], DT.float32, tag="acc2")
                nc.vector.memset(acc, 0.0)
                nc.vector.memset(acc2, 0.0)
                for gA in range(NGR):
                    gs = slice(gA * 4, (gA + 1) * 4)
                    nc.sync.dma_start(
                        out=sct[:, gs],
                        in_=scd[:, gA * 512:(gA + 1) * 512].rearrange(
                            "o (k t) -> (o t) k", t=TPC))
                    nc.vector.tensor_tensor(sct[:, gs], sct[:, gs],
                                            maskp[:, gs], op=ALU.add)
                    nc.scalar.activation(exv[:, gs], sct[:, gs], ACT.Exp)
                    for k in range(gA * 4, (gA + 1) * 4):
                        ac = acc if k % 2 == 0 else acc2
                        nc.vector.scalar_tensor_tensor(
                            ac, hk[k], exv[:, k:k + 1], ac,
                            op0=ALU.mult, op1=ALU.add)
                sm = cpool.tile([P, 1], DT.float32, tag="sm")
                nc.vector.reduce_sum(sm, exv, axis=mybir.AxisListType.X)
                rc = cpool.tile([P, 1], DT.float32, tag="rc")
                nc.vector.reciprocal(rc, sm)
                nc.vector.tensor_tensor(acc, acc, acc2, op=ALU.add)
                nc.vector.tensor_scalar(acc, acc, rc[:, :], None, op0=ALU.mult)
                nc.sync.dma_start(out=out, in_=acc)

    nc.compile()
    return nc


def get_nc():
    if "v6" not in _BUILD_CACHE:
        _BUILD_CACHE["v6"] = _build()
    return _BUILD_CACHE["v6"]


def kernel(conceptnet_text_vec, table, a, b, topk=16, **_ignored):
    global LAST_RESULTS
    assert int(topk) == TOPK
    tok = np.asarray(conceptnet_text_vec).reshape(NTOK, 1).astype(np.int32)
    table = np.ascontiguousarray(np.asarray(table, dtype=np.float32))
    a = np.ascontiguousarray(np.asarray(a, dtype=np.float32))
    b = np.ascontiguousarray(np.asarray(b, dtype=np.float32)).reshape(E, 1)
    tabT = np.ascontiguousarray(table.T)     # [E, V]

    nc = get_nc()
    in_maps = []
    for c in range(NCORES):
        in_maps.append({
            "tokidx": tok,
            "tok_own": np.ascontiguousarray(tok[c * TPC:(c + 1) * TPC]),
            "table": table,
            "tabTb": np.ascontiguousarray(
                tabT[:, c * VS:(c + 1) * VS]).astype(ml_dtypes.bfloat16),
            "amat": a,
            "bvec": b,
        })
    trace = bool(int(os.environ.get("CN_TRACE", "0")))
    res = bass_utils.run_bass_kernel_spmd(nc, in_maps, core_ids=list(range(NCORES)),
                                          trace=trace)
    LAST_RESULTS = res
    outp = np.concatenate([res.results[c]["out"] for c in range(NCORES)], axis=0)
    return outp.reshape(B, L, E)
